# revision 34
# baseline (speedup 1.0000x reference)
"""AuthorGroupAttention Trainium2 kernel.

Data-parallel over batch: 8 samples -> 8 NeuronCores, one sample per core.
Per-sample routing (reader_token) is resolved on the host by gathering the
routed per-group weights into per-core combined projection weights.

Device-side layout is fully transposed ([feature, token]) so every matmul
contracts with the contraction dim on partitions:
  - Q/K projections per head h produce [128=(d_gen|d_rdr), T] tiles from
    host-combined weights [Wq.T[:,h*64:] | RWq[g].T[:,h*64:]].
  - scores^T[s,t] via row-packed K=64 matmuls (gen rows 0-63, rdr 64-127).
  - exp on ScalarE directly from PSUM with scale=D**-0.5 folded in.
  - attention: stationary operand is a 192-wide per-head-pair block
    [v_even(64) | 1_e | 0*31 | 1_o | 0*31 | v_odd(64)]; even heads read
    cols 0:128 so attn lands in PSUM partitions 0-63 with the softmax
    denominator Z at row 64, odd heads read cols 64:192 so attn lands in
    partitions 64-127 with Z at row 32 (32-aligned as PSUM access needs).
    Every PSUM drain is then partition-aligned for DVE.
  - normalize+combine (0.45/Zg + 0.05/Zr) on VectorE: 1/Z computed by DVE
    reciprocal straight off the PSUM row, shifted to partition 0 by DMA,
    then broadcast to all partitions with the GPSIMD partition_broadcast
    custom op; output projection streams Wo.T per o-tile with the v-bias
    folded into the output bias on the host (probs rows sum to 0.5, so
    attention over (v + bv) contributes exactly 0.5*bv per row).
"""

import os
import sys

for _p in ("/opt/trn_rl_repo",):
    if os.path.isdir(_p) and _p not in sys.path:
        sys.path.insert(0, _p)

import numpy as np

import concourse.bass as bass
import concourse.mybir as mybir
from concourse import bacc
from concourse.tile import TileContext
from concourse.bass_utils import run_bass_kernel_spmd

B, T, E, H, G = 8, 1024, 1024, 16, 4
D = E // H  # 64
SCALING = float(D) ** -0.5
W_G = 0.9 / 2.0  # generic path weight after the /2
W_R = 0.1 / 2.0  # reader path weight after the /2

F32 = mybir.dt.float32
F32R = mybir.dt.float32r
EO = E // 128  # 8 e-tiles
SO = T // 128  # 8 s-tiles
OO = E // 128  # 8 o-tiles
VB = 192  # v block width per head pair


def build_nc():
    nc = bacc.Bacc(name="author_group_attention")

    hsT = nc.dram_tensor("hsT", [E, T], F32R, kind="ExternalInput")
    wq = nc.dram_tensor("wq", [E, H, 128], F32R, kind="ExternalInput")
    wk = nc.dram_tensor("wk", [E, H, 128], F32R, kind="ExternalInput")
    wv = nc.dram_tensor("wv", [E, E], F32R, kind="ExternalInput")
    wo = nc.dram_tensor("wo", [E, E], F32R, kind="ExternalInput")
    bqk = nc.dram_tensor("bqk", [128, 2 * H], F32, kind="ExternalInput")
    wcol = nc.dram_tensor("wcol", [128, 4], F32, kind="ExternalInput")
    bo = nc.dram_tensor("bo", [128, OO], F32, kind="ExternalInput")
    outT = nc.dram_tensor("outT", [E, T], F32, kind="ExternalOutput")

    with TileContext(nc) as tc:
        from contextlib import ExitStack

        with ExitStack() as stack:
            const = stack.enter_context(tc.tile_pool(name="const", bufs=1))
            ppsum = stack.enter_context(
                tc.tile_pool(name="ppsum", bufs=1, space="PSUM")
            )

            hsT_sb = const.tile([128, EO, T], F32R, tag="hsT")
            hsT_r = hsT.rearrange("(eo ep) t -> ep eo t", ep=128)
            # v blocks: [s_p, s_o, pair, 192] = [v_even |1| 0*62 |1| v_odd]
            v_sb = const.tile([128, SO, H // 2, VB], F32R, tag="v")
            U32 = mybir.dt.uint32
            ONE_F32_BITS = 0x3F800000
            nc.vector.memset(v_sb[:].bitcast(U32), 0)
            nc.vector.memset(v_sb[:, :, :, D].bitcast(U32), ONE_F32_BITS)
            nc.vector.memset(v_sb[:, :, :, 96].bitcast(U32), ONE_F32_BITS)
            comb_tiles = [
                const.tile([128, T], F32R, tag=f"comb{eo}", name=f"comb{eo}") for eo in range(EO)
            ]
            bqk_sb = const.tile([128, 2 * H], F32, tag="bqk")
            wcol_sb = const.tile([128, 4], F32, tag="wcol")
            bo_sb = const.tile([128, OO], F32, tag="bo")

            wpool = stack.enter_context(tc.tile_pool(name="wqk", bufs=3))
            qkpool = stack.enter_context(tc.tile_pool(name="qk", bufs=2))

            def proj_steps(h, which):
                """Projection of combined Q or K for head h as a list of
                emission steps, so the PE work can be interleaved into other
                loops. The weight DMA fires now."""
                wt = wpool.tile([128, EO, 128], F32R, tag="w")
                srcw = wq if which == "q" else wk
                nc.sync.dma_start(
                    wt[:], srcw[:, h, :].rearrange("(eo ep) m -> ep eo m", ep=128)
                )
                dst = qkpool.tile([128, T], F32R, tag=which)
                bias_col = 2 * h if which == "q" else 2 * h + 1
                state = {}

                def mk_mm(nh, eo):
                    def step():
                        if nh == 0 and eo == 0:
                            state[0] = ppsum.tile([128, T], F32, tag="proj", name="pq")
                        nc.tensor.matmul(
                            state[0][:, nh * 512 : (nh + 1) * 512],
                            wt[:, eo, :],
                            hsT_sb[:, eo, nh * 512 : (nh + 1) * 512],
                            start=(eo == 0),
                            stop=(eo == EO - 1),
                        )
                        if eo == EO - 1:
                            nc.vector.tensor_scalar_add(
                                dst[:, nh * 512 : (nh + 1) * 512],
                                state[0][:, nh * 512 : (nh + 1) * 512],
                                bqk_sb[:, bias_col : bias_col + 1],
                            )
                    return step

                return dst, [mk_mm(nh, eo) for nh in range(2) for eo in range(EO)]

            # ---------------- v projection (natural layout [s, o]) ---------
            with tc.tile_pool(name="wvp", bufs=1) as wvp, tc.tile_pool(
                name="vpsum", bufs=3, space="PSUM"
            ) as vpsum:
                wv_sb = wvp.tile([128, EO, E], F32R, tag="wv")
                wv_r = wv.rearrange("(eo ep) o -> ep eo o", ep=128)
                nc.sync.dma_start(bqk_sb[:], bqk[:])
                Qh, steps_q0 = proj_steps(0, "q")
                Kh, steps_k0 = proj_steps(0, "k")
                qk0_pump = steps_q0 + steps_k0
                for eo in range(EO):
                    nc.sync.dma_start(hsT_sb[:, eo], hsT_r[:, eo])
                    nc.sync.dma_start(wv_sb[:, eo], wv_r[:, eo])
                nc.sync.dma_start(wcol_sb[:], wcol[:])
                nc.sync.dma_start(bo_sb[:], bo[:])
                # s-tiles in groups of 3 with eo-inner emission: each arriving
                # (hsT, wv) chunk pair immediately feeds the whole group, which
                # keeps PE fed while the first chunks stream in
                for g0 in range(0, SO, 3):
                    grp = list(range(g0, min(g0 + 3, SO)))
                    pvs = {}
                    for so in grp:
                        pvs[so] = vpsum.tile([128, T], F32, tag="vproj",
                                             name=f"pv{so}")
                    for eo in range(EO):
                        for so in grp:
                            for nh in range(2):
                                nc.tensor.matmul(
                                    pvs[so][:, nh * 512 : (nh + 1) * 512],
                                    hsT_sb[:, eo, so * 128 : (so + 1) * 128],
                                    wv_sb[:, eo, nh * 512 : (nh + 1) * 512],
                                    start=(eo == 0),
                                    stop=(eo == EO - 1),
                                )
                        for _ in range(2):
                            if qk0_pump:
                                qk0_pump.pop(0)()
                    for so in grp:
                        pv4 = pvs[so].rearrange("p (m two d) -> p m two d", two=2, d=D)
                        nc.vector.tensor_copy(v_sb[:, so, :, 0:D], pv4[:, :, 0, :])
                        nc.vector.tensor_copy(
                            v_sb[:, so, :, 128 : 128 + D], pv4[:, :, 1, :]
                        )

                while qk0_pump:
                    qk0_pump.pop(0)()

            # ---------------- attention main loop ---------------------------
            with ExitStack() as attn_stack:
                expp = attn_stack.enter_context(tc.tile_pool(name="exp", bufs=4))
                rawp = attn_stack.enter_context(tc.tile_pool(name="raw", bufs=2))
                zp = attn_stack.enter_context(tc.tile_pool(name="z", bufs=1))
                bcp = attn_stack.enter_context(tc.tile_pool(name="bc", bufs=2))
                spsum = attn_stack.enter_context(
                    tc.tile_pool(name="spsum", bufs=2, space="PSUM")
                )
                apsum = attn_stack.enter_context(
                    tc.tile_pool(name="apsum", bufs=1, space="PSUM")
                )

                for h in range(H):
                    par_odd = h % 2  # 0 -> attn rows 0:64, Z row 64
                    abase = 64 * par_odd
                    zrow = 64 if par_odd == 0 else 32
                    voff = 64 * par_odd  # v block col offset
                    rawg = rawp.tile([128, T], F32, tag="rg")
                    rawr = rawp.tile([128, T], F32, tag="rr")
                    # zrec holds 1/Z rows (on partition zrow): cols 0:T gen,
                    # T:2T rdr
                    zrec = zp.tile([128, 2 * T], F32, tag="zrec")
                    nextQ = nextK = None
                    pump = []
                    if h + 1 < H:
                        nextQ, steps_q = proj_steps(h + 1, "q")
                        nextK, steps_k = proj_steps(h + 1, "k")
                        pump = steps_q + steps_k

                    for th in range(2):
                        tsl = slice(th * 512, (th + 1) * 512)
                        pag = apsum.tile([128, 512], F32, tag="ag")
                        par_ = apsum.tile([128, 512], F32, tag="ar")
                        for s in range(SO):
                            ps = spsum.tile([128, T], F32, tag="sc")
                            ssl = slice(s * 128, (s + 1) * 128)
                            nc.tensor.matmul(
                                ps[:, 0:512],
                                Kh[0:64, ssl],
                                Qh[0:64, tsl],
                                start=True,
                                stop=True,
                            )
                            nc.tensor.matmul(
                                ps[:, 512:1024],
                                Kh[64:128, ssl],
                                Qh[64:128, tsl],
                                start=True,
                                stop=True,
                            )
                            ex = expp.tile([128, T], F32R, tag="ex")
                            nc.scalar.activation(
                                ex[:],
                                ps[:],
                                mybir.ActivationFunctionType.Exp,
                                scale=SCALING,
                            )
                            vblk = v_sb[:, s, h // 2, voff : voff + 128]
                            nc.tensor.matmul(
                                pag[:],
                                vblk,
                                ex[:, 0:512],
                                start=(s == 0),
                                stop=(s == SO - 1),
                            )
                            nc.tensor.matmul(
                                par_[:],
                                vblk,
                                ex[:, 512:1024],
                                start=(s == 0),
                                stop=(s == SO - 1),
                            )
                            for _ in range(2):
                                if pump:
                                    pump.pop(0)()
                        # drain attention rows + Z row: wcol applies W^2 to
                        # attn rows and W to the Z row, so raw*(1/(W*Z))
                        # recovers W*attn/Z in the combine. Even heads merge
                        # both into one [0:65] op; odd heads need two ops
                        # (spans starting at partition 32 are limited to 32).
                        if par_odd == 0:
                            nc.vector.tensor_scalar_mul(
                                rawg[0:65, tsl], pag[0:65, :], wcol_sb[0:65, 0:1]
                            )
                            nc.vector.tensor_scalar_mul(
                                rawr[0:65, tsl], par_[0:65, :], wcol_sb[0:65, 1:2]
                            )
                        else:
                            nc.vector.tensor_scalar_mul(
                                rawg[64:128, tsl], pag[64:128, :], W_G * W_G
                            )
                            nc.vector.tensor_scalar_mul(
                                rawg[32:33, tsl], pag[32:33, :], W_G
                            )
                            nc.vector.tensor_scalar_mul(
                                rawr[64:128, tsl], par_[64:128, :], W_R * W_R
                            )
                            nc.vector.tensor_scalar_mul(
                                rawr[32:33, tsl], par_[32:33, :], W_R
                            )
                        zsl = slice(zrow, zrow + 1)
                        nc.vector.reciprocal(
                            zrec[zsl, th * 512 : th * 512 + 512], rawg[zsl, tsl]
                        )
                        nc.vector.reciprocal(
                            zrec[zsl, T + th * 512 : T + th * 512 + 512],
                            rawr[zsl, tsl],
                        )
                        # shift 1/Z rows to partition 0 (DMA can cross
                        # partitions), broadcast on GPSIMD, combine this half
                        nc.sync.dma_start(
                            zrec[0:1, th * 512 : th * 512 + 512],
                            zrec[zrow : zrow + 1, th * 512 : th * 512 + 512],
                        )
                        nc.sync.dma_start(
                            zrec[0:1, T + th * 512 : T + th * 512 + 512],
                            zrec[zrow : zrow + 1, T + th * 512 : T + th * 512 + 512],
                        )
                        bcg = bcp.tile([128, 512], F32, tag="bg")
                        bcr = bcp.tile([128, 512], F32, tag="br")
                        nc.gpsimd.partition_broadcast(
                            bcg[:], zrec[0:1, th * 512 : th * 512 + 512]
                        )
                        nc.gpsimd.partition_broadcast(
                            bcr[:], zrec[0:1, T + th * 512 : T + th * 512 + 512]
                        )
                        asl2 = slice(abase, abase + 64)
                        nc.vector.tensor_mul(
                            rawg[asl2, tsl], rawg[asl2, tsl], bcg[asl2, :]
                        )
                        nc.vector.tensor_mul(
                            rawr[asl2, tsl], rawr[asl2, tsl], bcr[asl2, :]
                        )
                        nc.vector.tensor_add(
                            comb_tiles[h // 2][asl2, tsl],
                            rawg[asl2, tsl],
                            rawr[asl2, tsl],
                        )
                    while pump:
                        pump.pop(0)()
                    if h + 1 < H:
                        Qh, Kh = nextQ, nextK

            # ---------------- output projection -----------------------------
            with tc.tile_pool(name="tail", bufs=3) as tailp, tc.tile_pool(
                name="outsb", bufs=2
            ) as outp, tc.tile_pool(name="opsum", bufs=2, space="PSUM") as opsum:
                wo_r = wo.rearrange("(eo ep) (oo m) -> oo ep eo m", ep=128, m=128)
                for j in range(OO):
                    wt = tailp.tile([128, EO, 128], F32R, tag="wo")
                    nc.sync.dma_start(wt[:], wo_r[j])
                    po = opsum.tile([128, T], F32, tag="oproj")
                    ot = outp.tile([128, T], F32, tag="ot")
                    for nh in range(2):
                        for eo in range(EO):
                            nc.tensor.matmul(
                                po[:, nh * 512 : (nh + 1) * 512],
                                wt[:, eo, :],
                                comb_tiles[eo][:, nh * 512 : (nh + 1) * 512],
                                start=(eo == 0),
                                stop=(eo == EO - 1),
                            )
                        nc.vector.tensor_scalar_add(
                            ot[:, nh * 512 : (nh + 1) * 512],
                            po[:, nh * 512 : (nh + 1) * 512],
                            bo_sb[:, j : j + 1],
                        )
                        nc.sync.dma_start(
                            outT[j * 128 : (j + 1) * 128, nh * 512 : (nh + 1) * 512],
                            ot[:, nh * 512 : (nh + 1) * 512],
                        )

    nc.finalize()
    return nc


_NC_CACHE = {}


def get_nc():
    if "nc" not in _NC_CACHE:
        _NC_CACHE["nc"] = build_nc()
    return _NC_CACHE["nc"]


def _host_prep(hidden_states, reader_token, Wq, bq, Wk, bk, Wv, bv, Wo, bo,
               RWq, Rbq, RWk, Rbk, RWv, Rbv):
    """Build the 8 per-core input maps (numpy only)."""
    f = np.float32
    hs = np.asarray(hidden_states, f)
    tok = np.asarray(reader_token).astype(np.int64)
    WqT = np.ascontiguousarray(np.asarray(Wq, f).T)  # [e, o]
    WkT = np.ascontiguousarray(np.asarray(Wk, f).T)
    WvT = np.ascontiguousarray(np.asarray(Wv, f).T)
    WoT = np.ascontiguousarray(np.asarray(Wo, f).T)
    RWqT = np.transpose(np.asarray(RWq, f), (0, 2, 1))  # [g, e, o]
    RWkT = np.transpose(np.asarray(RWk, f), (0, 2, 1))
    bq = np.asarray(bq, f); bk = np.asarray(bk, f)
    bv = np.asarray(bv, f); bo_ = np.asarray(bo, f)
    Rbq = np.asarray(Rbq, f); Rbk = np.asarray(Rbk, f)

    # v-bias folds into the output bias: probs rows sum to 0.5, so attention
    # over (v + bv) adds 0.5*bv to every attn row -> out += 0.5 * bv @ Wo.T
    bo_eff = bo_ + 0.5 * (np.asarray(Wo, f) @ bv)
    bo_t = np.ascontiguousarray(bo_eff.reshape(OO, 128).T)  # [128, oo]

    # shared [e, h, 64] views of the generic weights
    WqT_h = WqT.reshape(E, H, D)
    WkT_h = WkT.reshape(E, H, D)

    wcol_t = np.zeros((128, 4), f)
    wcol_t[0:64, 0] = W_G * W_G
    wcol_t[64, 0] = W_G
    wcol_t[0:64, 1] = W_R * W_R
    wcol_t[64, 1] = W_R
    wcol_t[64:128, 2] = W_G * W_G
    wcol_t[32, 2] = W_G
    wcol_t[64:128, 3] = W_R * W_R
    wcol_t[32, 3] = W_R

    in_maps = []
    percore = {}
    for b in range(B):
        g = int(tok[b])
        if g not in percore:
            wqc = np.empty((E, H, 128), f)
            wqc[:, :, :D] = WqT_h
            wqc[:, :, D:] = RWqT[g].reshape(E, H, D)
            wkc = np.empty((E, H, 128), f)
            wkc[:, :, :D] = WkT_h
            wkc[:, :, D:] = RWkT[g].reshape(E, H, D)
            # per-head combined biases: col 2h = [bq_h|Rbq_h], col 2h+1 = k
            bqk_t = np.empty((128, 2 * H), f)
            bqk_t[:D, 0::2] = bq.reshape(H, D).T
            bqk_t[D:, 0::2] = Rbq[g].reshape(H, D).T
            bqk_t[:D, 1::2] = bk.reshape(H, D).T
            bqk_t[D:, 1::2] = Rbk[g].reshape(H, D).T
            percore[g] = (wqc, wkc, bqk_t)
        wqc, wkc, bqk_t = percore[g]
        in_maps.append(
            {
                "hsT": np.ascontiguousarray(hs[b].T),
                "wq": wqc,
                "wk": wkc,
                "wv": WvT,
                "wo": WoT,
                "bqk": bqk_t,
                "wcol": wcol_t,
                "bo": bo_t,
            }
        )
    return in_maps


def kernel(**inputs) -> np.ndarray:
    in_maps = _host_prep(**inputs)
    nc = get_nc()
    res = run_bass_kernel_spmd(nc, in_maps, list(range(B)))
    out = np.stack([res.results[c]["outT"].T for c in range(B)], axis=0)
    return np.ascontiguousarray(out.astype(np.float32))


if __name__ == "__main__":
    rng = np.random.default_rng(0)
    ins = {
        "hidden_states": rng.standard_normal((B, T, E), dtype=np.float32),
        "reader_token": rng.integers(0, G, size=(B,)).astype(np.int32),
        "Wq": rng.standard_normal((E, E), dtype=np.float32) * 0.02,
        "bq": np.zeros(E, np.float32),
        "Wk": rng.standard_normal((E, E), dtype=np.float32) * 0.02,
        "bk": np.zeros(E, np.float32),
        "Wv": rng.standard_normal((E, E), dtype=np.float32) * 0.02,
        "bv": np.zeros(E, np.float32),
        "Wo": rng.standard_normal((E, E), dtype=np.float32) * 0.02,
        "bo": np.zeros(E, np.float32),
        "RWq": rng.standard_normal((G, E, E), dtype=np.float32) * 0.02,
        "Rbq": np.zeros((G, E), np.float32),
        "RWk": rng.standard_normal((G, E, E), dtype=np.float32) * 0.02,
        "Rbk": np.zeros((G, E), np.float32),
        "RWv": rng.standard_normal((G, E, E), dtype=np.float32) * 0.02,
        "Rbv": np.zeros((G, E), np.float32),
    }
    out = kernel(**ins)
    print("out", out.shape, out.dtype, float(np.abs(out).max()))


# revision 35
# speedup vs baseline: 1.0032x; 1.0032x over previous
"""AuthorGroupAttention Trainium2 kernel.

Data-parallel over batch: 8 samples -> 8 NeuronCores, one sample per core.
Per-sample routing (reader_token) is resolved on the host by gathering the
routed per-group weights into per-core combined projection weights.

Device-side layout is fully transposed ([feature, token]) so every matmul
contracts with the contraction dim on partitions:
  - Q/K projections per head h produce [128=(d_gen|d_rdr), T] tiles from
    host-combined weights [Wq.T[:,h*64:] | RWq[g].T[:,h*64:]].
  - scores^T[s,t] via row-packed K=64 matmuls (gen rows 0-63, rdr 64-127).
  - exp on ScalarE directly from PSUM with scale=D**-0.5 folded in.
  - attention: stationary operand is a 192-wide per-head-pair block
    [v_even(64) | 1_e | 0*31 | 1_o | 0*31 | v_odd(64)]; even heads read
    cols 0:128 so attn lands in PSUM partitions 0-63 with the softmax
    denominator Z at row 64, odd heads read cols 64:192 so attn lands in
    partitions 64-127 with Z at row 32 (32-aligned as PSUM access needs).
    Every PSUM drain is then partition-aligned for DVE.
  - normalize+combine (0.45/Zg + 0.05/Zr) on VectorE: 1/Z computed by DVE
    reciprocal straight off the PSUM row, shifted to partition 0 by DMA,
    then broadcast to all partitions with the GPSIMD partition_broadcast
    custom op; output projection streams Wo.T per o-tile with the v-bias
    folded into the output bias on the host (probs rows sum to 0.5, so
    attention over (v + bv) contributes exactly 0.5*bv per row).
"""

import os
import sys

for _p in ("/opt/trn_rl_repo",):
    if os.path.isdir(_p) and _p not in sys.path:
        sys.path.insert(0, _p)

import numpy as np

import concourse.bass as bass
import concourse.mybir as mybir
from concourse import bacc
from concourse.tile import TileContext
from concourse.bass_utils import run_bass_kernel_spmd

B, T, E, H, G = 8, 1024, 1024, 16, 4
D = E // H  # 64
SCALING = float(D) ** -0.5
W_G = 0.9 / 2.0  # generic path weight after the /2
W_R = 0.1 / 2.0  # reader path weight after the /2

F32 = mybir.dt.float32
F32R = mybir.dt.float32r
EO = E // 128  # 8 e-tiles
SO = T // 128  # 8 s-tiles
OO = E // 128  # 8 o-tiles
VB = 192  # v block width per head pair


def build_nc():
    nc = bacc.Bacc(name="author_group_attention")

    hsT = nc.dram_tensor("hsT", [E, T], F32R, kind="ExternalInput")
    wq = nc.dram_tensor("wq", [E, H, 128], F32R, kind="ExternalInput")
    wk = nc.dram_tensor("wk", [E, H, 128], F32R, kind="ExternalInput")
    wv = nc.dram_tensor("wv", [E, E], F32R, kind="ExternalInput")
    wo = nc.dram_tensor("wo", [E, E], F32R, kind="ExternalInput")
    bqk = nc.dram_tensor("bqk", [128, 2 * H], F32, kind="ExternalInput")
    wcol = nc.dram_tensor("wcol", [128, 4], F32, kind="ExternalInput")
    bo = nc.dram_tensor("bo", [128, OO], F32, kind="ExternalInput")
    outT = nc.dram_tensor("outT", [E, T], F32, kind="ExternalOutput")

    with TileContext(nc) as tc:
        from contextlib import ExitStack

        with ExitStack() as stack:
            const = stack.enter_context(tc.tile_pool(name="const", bufs=1))
            ppsum = stack.enter_context(
                tc.tile_pool(name="ppsum", bufs=1, space="PSUM")
            )

            hsT_sb = const.tile([128, EO, T], F32R, tag="hsT")
            hsT_r = hsT.rearrange("(eo ep) t -> ep eo t", ep=128)
            # v blocks: [s_p, s_o, pair, 192] = [v_even |1| 0*62 |1| v_odd]
            v_sb = const.tile([128, SO, H // 2, VB], F32R, tag="v")
            U32 = mybir.dt.uint32
            ONE_F32_BITS = 0x3F800000
            nc.vector.memset(v_sb[:].bitcast(U32), 0)
            nc.vector.memset(v_sb[:, :, :, D].bitcast(U32), ONE_F32_BITS)
            nc.vector.memset(v_sb[:, :, :, 96].bitcast(U32), ONE_F32_BITS)
            comb_tiles = [
                const.tile([128, T], F32R, tag=f"comb{eo}", name=f"comb{eo}") for eo in range(EO)
            ]
            bqk_sb = const.tile([128, 2 * H], F32, tag="bqk")
            wcol_sb = const.tile([128, 4], F32, tag="wcol")
            bo_sb = const.tile([128, OO], F32, tag="bo")

            wpool = stack.enter_context(tc.tile_pool(name="wqk", bufs=3))
            qkpool = stack.enter_context(tc.tile_pool(name="qk", bufs=2))

            def proj_steps(h, which):
                """Projection of combined Q or K for head h as a list of
                emission steps, so the PE work can be interleaved into other
                loops. The weight DMA fires now."""
                wt = wpool.tile([128, EO, 128], F32R, tag="w")
                srcw = wq if which == "q" else wk
                nc.sync.dma_start(
                    wt[:], srcw[:, h, :].rearrange("(eo ep) m -> ep eo m", ep=128)
                )
                dst = qkpool.tile([128, T], F32R, tag=which)
                bias_col = 2 * h if which == "q" else 2 * h + 1
                state = {}

                def mk_mm(nh, eo):
                    def step():
                        if nh == 0 and eo == 0:
                            state[0] = ppsum.tile([128, T], F32, tag="proj", name="pq")
                        nc.tensor.matmul(
                            state[0][:, nh * 512 : (nh + 1) * 512],
                            wt[:, eo, :],
                            hsT_sb[:, eo, nh * 512 : (nh + 1) * 512],
                            start=(eo == 0),
                            stop=(eo == EO - 1),
                        )
                        if eo == EO - 1:
                            nc.vector.tensor_scalar_add(
                                dst[:, nh * 512 : (nh + 1) * 512],
                                state[0][:, nh * 512 : (nh + 1) * 512],
                                bqk_sb[:, bias_col : bias_col + 1],
                            )
                    return step

                return dst, [mk_mm(nh, eo) for nh in range(2) for eo in range(EO)]

            # ---------------- v projection (natural layout [s, o]) ---------
            with tc.tile_pool(name="wvp", bufs=1) as wvp, tc.tile_pool(
                name="vpsum", bufs=3, space="PSUM"
            ) as vpsum:
                wv_sb = wvp.tile([128, EO, E], F32R, tag="wv")
                wv_r = wv.rearrange("(eo ep) o -> ep eo o", ep=128)
                nc.sync.dma_start(bqk_sb[:], bqk[:])
                Qh, steps_q0 = proj_steps(0, "q")
                Kh, steps_k0 = proj_steps(0, "k")
                qk0_pump = steps_q0 + steps_k0
                for eo in range(EO):
                    nc.sync.dma_start(hsT_sb[:, eo], hsT_r[:, eo])
                    nc.sync.dma_start(wv_sb[:, eo], wv_r[:, eo])
                nc.sync.dma_start(wcol_sb[:], wcol[:])
                nc.sync.dma_start(bo_sb[:], bo[:])
                # s-tiles in groups of 3 with eo-inner emission: each arriving
                # (hsT, wv) chunk pair immediately feeds the whole group, which
                # keeps PE fed while the first chunks stream in
                for g0 in range(0, SO, 3):
                    grp = list(range(g0, min(g0 + 3, SO)))
                    pvs = {}
                    for so in grp:
                        pvs[so] = vpsum.tile([128, T], F32, tag="vproj",
                                             name=f"pv{so}")
                    for eo in range(EO):
                        for so in grp:
                            for nh in range(2):
                                nc.tensor.matmul(
                                    pvs[so][:, nh * 512 : (nh + 1) * 512],
                                    hsT_sb[:, eo, so * 128 : (so + 1) * 128],
                                    wv_sb[:, eo, nh * 512 : (nh + 1) * 512],
                                    start=(eo == 0),
                                    stop=(eo == EO - 1),
                                )
                        for _ in range(2 if g0 >= 6 else 1):
                            if qk0_pump:
                                qk0_pump.pop(0)()
                    for so in grp:
                        pv4 = pvs[so].rearrange("p (m two d) -> p m two d", two=2, d=D)
                        nc.vector.tensor_copy(v_sb[:, so, :, 0:D], pv4[:, :, 0, :])
                        nc.vector.tensor_copy(
                            v_sb[:, so, :, 128 : 128 + D], pv4[:, :, 1, :]
                        )

                while qk0_pump:
                    qk0_pump.pop(0)()

            # ---------------- attention main loop ---------------------------
            with ExitStack() as attn_stack:
                expp = attn_stack.enter_context(tc.tile_pool(name="exp", bufs=4))
                rawp = attn_stack.enter_context(tc.tile_pool(name="raw", bufs=2))
                zp = attn_stack.enter_context(tc.tile_pool(name="z", bufs=1))
                bcp = attn_stack.enter_context(tc.tile_pool(name="bc", bufs=2))
                spsum = attn_stack.enter_context(
                    tc.tile_pool(name="spsum", bufs=2, space="PSUM")
                )
                apsum = attn_stack.enter_context(
                    tc.tile_pool(name="apsum", bufs=1, space="PSUM")
                )

                for h in range(H):
                    par_odd = h % 2  # 0 -> attn rows 0:64, Z row 64
                    abase = 64 * par_odd
                    zrow = 64 if par_odd == 0 else 32
                    voff = 64 * par_odd  # v block col offset
                    rawg = rawp.tile([128, T], F32, tag="rg")
                    rawr = rawp.tile([128, T], F32, tag="rr")
                    # zrec holds 1/Z rows (on partition zrow): cols 0:T gen,
                    # T:2T rdr
                    zrec = zp.tile([128, 2 * T], F32, tag="zrec")
                    nextQ = nextK = None
                    pump = []
                    if h + 1 < H:
                        nextQ, steps_q = proj_steps(h + 1, "q")
                        nextK, steps_k = proj_steps(h + 1, "k")
                        pump = steps_q + steps_k

                    for th in range(2):
                        tsl = slice(th * 512, (th + 1) * 512)
                        pag = apsum.tile([128, 512], F32, tag="ag")
                        par_ = apsum.tile([128, 512], F32, tag="ar")
                        for s in range(SO):
                            ps = spsum.tile([128, T], F32, tag="sc")
                            ssl = slice(s * 128, (s + 1) * 128)
                            nc.tensor.matmul(
                                ps[:, 0:512],
                                Kh[0:64, ssl],
                                Qh[0:64, tsl],
                                start=True,
                                stop=True,
                            )
                            nc.tensor.matmul(
                                ps[:, 512:1024],
                                Kh[64:128, ssl],
                                Qh[64:128, tsl],
                                start=True,
                                stop=True,
                            )
                            ex = expp.tile([128, T], F32R, tag="ex")
                            nc.scalar.activation(
                                ex[:],
                                ps[:],
                                mybir.ActivationFunctionType.Exp,
                                scale=SCALING,
                            )
                            vblk = v_sb[:, s, h // 2, voff : voff + 128]
                            nc.tensor.matmul(
                                pag[:],
                                vblk,
                                ex[:, 0:512],
                                start=(s == 0),
                                stop=(s == SO - 1),
                            )
                            nc.tensor.matmul(
                                par_[:],
                                vblk,
                                ex[:, 512:1024],
                                start=(s == 0),
                                stop=(s == SO - 1),
                            )
                            for _ in range(2):
                                if pump:
                                    pump.pop(0)()
                        # drain attention rows + Z row: wcol applies W^2 to
                        # attn rows and W to the Z row, so raw*(1/(W*Z))
                        # recovers W*attn/Z in the combine. Even heads merge
                        # both into one [0:65] op; odd heads need two ops
                        # (spans starting at partition 32 are limited to 32).
                        if par_odd == 0:
                            nc.vector.tensor_scalar_mul(
                                rawg[0:65, tsl], pag[0:65, :], wcol_sb[0:65, 0:1]
                            )
                            nc.vector.tensor_scalar_mul(
                                rawr[0:65, tsl], par_[0:65, :], wcol_sb[0:65, 1:2]
                            )
                        else:
                            nc.vector.tensor_scalar_mul(
                                rawg[64:128, tsl], pag[64:128, :], W_G * W_G
                            )
                            nc.vector.tensor_scalar_mul(
                                rawg[32:33, tsl], pag[32:33, :], W_G
                            )
                            nc.vector.tensor_scalar_mul(
                                rawr[64:128, tsl], par_[64:128, :], W_R * W_R
                            )
                            nc.vector.tensor_scalar_mul(
                                rawr[32:33, tsl], par_[32:33, :], W_R
                            )
                        zsl = slice(zrow, zrow + 1)
                        nc.vector.reciprocal(
                            zrec[zsl, th * 512 : th * 512 + 512], rawg[zsl, tsl]
                        )
                        nc.vector.reciprocal(
                            zrec[zsl, T + th * 512 : T + th * 512 + 512],
                            rawr[zsl, tsl],
                        )
                        # shift 1/Z rows to partition 0 (DMA can cross
                        # partitions), broadcast on GPSIMD, combine this half
                        nc.sync.dma_start(
                            zrec[0:1, th * 512 : th * 512 + 512],
                            zrec[zrow : zrow + 1, th * 512 : th * 512 + 512],
                        )
                        nc.sync.dma_start(
                            zrec[0:1, T + th * 512 : T + th * 512 + 512],
                            zrec[zrow : zrow + 1, T + th * 512 : T + th * 512 + 512],
                        )
                        bcg = bcp.tile([128, 512], F32, tag="bg")
                        bcr = bcp.tile([128, 512], F32, tag="br")
                        nc.gpsimd.partition_broadcast(
                            bcg[:], zrec[0:1, th * 512 : th * 512 + 512]
                        )
                        nc.gpsimd.partition_broadcast(
                            bcr[:], zrec[0:1, T + th * 512 : T + th * 512 + 512]
                        )
                        asl2 = slice(abase, abase + 64)
                        nc.vector.tensor_mul(
                            rawg[asl2, tsl], rawg[asl2, tsl], bcg[asl2, :]
                        )
                        nc.vector.tensor_mul(
                            rawr[asl2, tsl], rawr[asl2, tsl], bcr[asl2, :]
                        )
                        nc.vector.tensor_add(
                            comb_tiles[h // 2][asl2, tsl],
                            rawg[asl2, tsl],
                            rawr[asl2, tsl],
                        )
                    while pump:
                        pump.pop(0)()
                    if h + 1 < H:
                        Qh, Kh = nextQ, nextK

            # ---------------- output projection -----------------------------
            with tc.tile_pool(name="tail", bufs=3) as tailp, tc.tile_pool(
                name="outsb", bufs=2
            ) as outp, tc.tile_pool(name="opsum", bufs=2, space="PSUM") as opsum:
                wo_r = wo.rearrange("(eo ep) (oo m) -> oo ep eo m", ep=128, m=128)
                for j in range(OO):
                    wt = tailp.tile([128, EO, 128], F32R, tag="wo")
                    nc.sync.dma_start(wt[:], wo_r[j])
                    po = opsum.tile([128, T], F32, tag="oproj")
                    ot = outp.tile([128, T], F32, tag="ot")
                    for nh in range(2):
                        for eo in range(EO):
                            nc.tensor.matmul(
                                po[:, nh * 512 : (nh + 1) * 512],
                                wt[:, eo, :],
                                comb_tiles[eo][:, nh * 512 : (nh + 1) * 512],
                                start=(eo == 0),
                                stop=(eo == EO - 1),
                            )
                        nc.vector.tensor_scalar_add(
                            ot[:, nh * 512 : (nh + 1) * 512],
                            po[:, nh * 512 : (nh + 1) * 512],
                            bo_sb[:, j : j + 1],
                        )
                        nc.sync.dma_start(
                            outT[j * 128 : (j + 1) * 128, nh * 512 : (nh + 1) * 512],
                            ot[:, nh * 512 : (nh + 1) * 512],
                        )

    nc.finalize()
    return nc


_NC_CACHE = {}


def get_nc():
    if "nc" not in _NC_CACHE:
        _NC_CACHE["nc"] = build_nc()
    return _NC_CACHE["nc"]


def _host_prep(hidden_states, reader_token, Wq, bq, Wk, bk, Wv, bv, Wo, bo,
               RWq, Rbq, RWk, Rbk, RWv, Rbv):
    """Build the 8 per-core input maps (numpy only)."""
    f = np.float32
    hs = np.asarray(hidden_states, f)
    tok = np.asarray(reader_token).astype(np.int64)
    WqT = np.ascontiguousarray(np.asarray(Wq, f).T)  # [e, o]
    WkT = np.ascontiguousarray(np.asarray(Wk, f).T)
    WvT = np.ascontiguousarray(np.asarray(Wv, f).T)
    WoT = np.ascontiguousarray(np.asarray(Wo, f).T)
    RWqT = np.transpose(np.asarray(RWq, f), (0, 2, 1))  # [g, e, o]
    RWkT = np.transpose(np.asarray(RWk, f), (0, 2, 1))
    bq = np.asarray(bq, f); bk = np.asarray(bk, f)
    bv = np.asarray(bv, f); bo_ = np.asarray(bo, f)
    Rbq = np.asarray(Rbq, f); Rbk = np.asarray(Rbk, f)

    # v-bias folds into the output bias: probs rows sum to 0.5, so attention
    # over (v + bv) adds 0.5*bv to every attn row -> out += 0.5 * bv @ Wo.T
    bo_eff = bo_ + 0.5 * (np.asarray(Wo, f) @ bv)
    bo_t = np.ascontiguousarray(bo_eff.reshape(OO, 128).T)  # [128, oo]

    # shared [e, h, 64] views of the generic weights
    WqT_h = WqT.reshape(E, H, D)
    WkT_h = WkT.reshape(E, H, D)

    wcol_t = np.zeros((128, 4), f)
    wcol_t[0:64, 0] = W_G * W_G
    wcol_t[64, 0] = W_G
    wcol_t[0:64, 1] = W_R * W_R
    wcol_t[64, 1] = W_R
    wcol_t[64:128, 2] = W_G * W_G
    wcol_t[32, 2] = W_G
    wcol_t[64:128, 3] = W_R * W_R
    wcol_t[32, 3] = W_R

    in_maps = []
    percore = {}
    for b in range(B):
        g = int(tok[b])
        if g not in percore:
            wqc = np.empty((E, H, 128), f)
            wqc[:, :, :D] = WqT_h
            wqc[:, :, D:] = RWqT[g].reshape(E, H, D)
            wkc = np.empty((E, H, 128), f)
            wkc[:, :, :D] = WkT_h
            wkc[:, :, D:] = RWkT[g].reshape(E, H, D)
            # per-head combined biases: col 2h = [bq_h|Rbq_h], col 2h+1 = k
            bqk_t = np.empty((128, 2 * H), f)
            bqk_t[:D, 0::2] = bq.reshape(H, D).T
            bqk_t[D:, 0::2] = Rbq[g].reshape(H, D).T
            bqk_t[:D, 1::2] = bk.reshape(H, D).T
            bqk_t[D:, 1::2] = Rbk[g].reshape(H, D).T
            percore[g] = (wqc, wkc, bqk_t)
        wqc, wkc, bqk_t = percore[g]
        in_maps.append(
            {
                "hsT": np.ascontiguousarray(hs[b].T),
                "wq": wqc,
                "wk": wkc,
                "wv": WvT,
                "wo": WoT,
                "bqk": bqk_t,
                "wcol": wcol_t,
                "bo": bo_t,
            }
        )
    return in_maps


def kernel(**inputs) -> np.ndarray:
    in_maps = _host_prep(**inputs)
    nc = get_nc()
    res = run_bass_kernel_spmd(nc, in_maps, list(range(B)))
    out = np.stack([res.results[c]["outT"].T for c in range(B)], axis=0)
    return np.ascontiguousarray(out.astype(np.float32))


if __name__ == "__main__":
    rng = np.random.default_rng(0)
    ins = {
        "hidden_states": rng.standard_normal((B, T, E), dtype=np.float32),
        "reader_token": rng.integers(0, G, size=(B,)).astype(np.int32),
        "Wq": rng.standard_normal((E, E), dtype=np.float32) * 0.02,
        "bq": np.zeros(E, np.float32),
        "Wk": rng.standard_normal((E, E), dtype=np.float32) * 0.02,
        "bk": np.zeros(E, np.float32),
        "Wv": rng.standard_normal((E, E), dtype=np.float32) * 0.02,
        "bv": np.zeros(E, np.float32),
        "Wo": rng.standard_normal((E, E), dtype=np.float32) * 0.02,
        "bo": np.zeros(E, np.float32),
        "RWq": rng.standard_normal((G, E, E), dtype=np.float32) * 0.02,
        "Rbq": np.zeros((G, E), np.float32),
        "RWk": rng.standard_normal((G, E, E), dtype=np.float32) * 0.02,
        "Rbk": np.zeros((G, E), np.float32),
        "RWv": rng.standard_normal((G, E, E), dtype=np.float32) * 0.02,
        "Rbv": np.zeros((G, E), np.float32),
    }
    out = kernel(**ins)
    print("out", out.shape, out.dtype, float(np.abs(out).max()))
